# revision 56
# baseline (speedup 1.0000x reference)
"""Trainium2 Bass kernel for nn_AttentionConv (sparse checkerboard attention).

Math (per batch image, C=64, H=W=32, N=4096 upsampled tokens):
  q,k,v = 1x1 convs; q is bilinearly 2x-upsampled, k/v zero-upsampled
  (values only at (even,even) positions).  A checkerboard mask of -1e8 is
  added to k itself, so the 3072 masked key columns are all identically
  (-1e8,...,-1e8): their score for query n is -1e8*S(n) with
  S(n)=sum_d q_up[n,d], and their v is 0.  Hence
     out[c,n] = sum_{m' in 1024 unmasked} v[c,m'] exp(s[n,m']) / D(n)
     D(n)     = big*(S(n)<=0) + sum_{m'} exp(s[n,m'])
  with s[n,m'] = q_up[n,:].k[:,m'].  Unmasked scores are O(40) so exp is
  computed without max-subtraction; rows with S<=0 get a 1e30 denominator
  term which drives the row to ~1e-11 (reference: exactly 0).

Sharding: 8 cores = 2 batches x 4 query-slices of 1024 tokens
(16 upsampled rows each).  No collectives; each core writes a disjoint
[64, 1024] output slice.

Schedule (v2): f32r everywhere precision matters (q/k chain, scores);
bf16 for exp output and the PV matmuls.  The q pipeline is
proj -> row-interp (kron(Ah-block, I32) matmul) -> col-interp (row-group
packed).  The per-core row-window of the bilinear interp is baked into
per-core DATA (host-gathered xw + kron matrices) so the instruction
stream is identical across cores (SPMD).  Dummy matmuls warm the PE
clock gate during the input DMA; a dummy exp preloads the ACT table.
"""
import math
import os
import sys

import numpy as np

if "/opt/trn_rl_repo" not in sys.path:
    sys.path.insert(0, "/opt/trn_rl_repo")

B, C, H, W = 2, 64, 32, 32
D = 8          # q/k head dim
NQ = 1024      # query tokens per core (16 upsampled rows x 64 cols)
NK = 1024      # unmasked keys per image (= H*W)
N_CORES = 8
MASK_BIG = 1.0e30  # masked-row denominator (kept < 1e38 for approx recip)


def _interp_consts():
    # float32 replica of reference's bilinear (align_corners=True) positions
    pos = np.arange(2 * H, dtype=np.float32) * np.float32((H - 1) / (2 * H - 1))
    i0 = np.clip(np.floor(pos), 0, H - 2).astype(np.int32)
    w = (pos - i0.astype(np.float32)).astype(np.float32)
    return pos, i0, w


def _row_windows(S):
    """For core query-slice S: per sub-slice s (4 up-rows each), the 4-row
    input window h0 and the 4x4 coefficient block A4[i', hh]."""
    _, i0, w = _interp_consts()
    out = []
    for s in range(4):
        rows = [16 * S + 4 * s + ii for ii in range(4)]
        h_lo = min(int(i0[r]) for r in rows)
        h0 = min(h_lo, H - 4)
        assert max(int(i0[r]) + 1 for r in rows) < h0 + 4
        A4 = np.zeros((4, 4), np.float32)
        for ii, r in enumerate(rows):
            A4[ii, int(i0[r]) - h0] += np.float32(1.0) - w[r]
            A4[ii, int(i0[r]) + 1 - h0] += w[r]
        out.append((h0, A4))
    return out


def _col_mat():
    # Block-diagonal Aw^T: awT4w[32i' + w, 64i' + J] = Aw[J, w].  One matmul
    # per 4-row slice then computes all 4 up-rows' col-interp at N=256.
    pos, i0, w = _interp_consts()
    A = np.zeros((2 * W, W), np.float32)
    r = np.arange(2 * W)
    np.add.at(A, (r, i0), 1.0 - w)
    np.add.at(A, (r, i0 + 1), w)
    AT = np.ascontiguousarray(A.T)  # [32, 64]
    out = np.zeros((128, 256), np.float32)
    for ip in range(4):
        out[32 * ip : 32 * (ip + 1), 64 * ip : 64 * (ip + 1)] = AT
    return out


def _build_nc():
    import concourse.bacc as bacc
    import concourse.mybir as mybir
    from concourse import tile

    f32 = mybir.dt.float32
    f32r = mybir.dt.float32r
    bf16 = mybir.dt.bfloat16
    EXP = mybir.ActivationFunctionType.Exp

    nc = bacc.Bacc(None, target_bir_lowering=False)

    xb_e = nc.declare_dram_parameter("xb", [C, NK], f32r, isOutput=False)
    c64_e = nc.declare_dram_parameter("c64", [C, 872], f32r, isOutput=False)
    c128_e = nc.declare_dram_parameter("c128", [128, 833], f32r, isOutput=False)
    out_e = nc.declare_dram_parameter("out", [C, NQ], f32, isOutput=True)

    with tile.TileContext(nc) as tc:
        with (
            nc.allow_low_precision(
                reason="bf16 PV accumulation + approx reciprocal are within "
                "the 2e-2 tolerance; q/k/score chain stays f32r"
            ),
            tc.tile_pool(name="const", bufs=1) as cst,
            tc.tile_pool(name="sb", bufs=1) as sbp,
            tc.tile_pool(name="pexp", bufs=3) as pexp,
        ):
            # ---- constants / inputs (one DMA per queue) ----
            xb = cst.tile([C, NK], f32r)
            nc.sync.dma_start(xb[:], xb_e[:])
            c64 = cst.tile([C, 872], f32r)
            nc.scalar.dma_start(c64[:, 0:768], c64_e[:, 0:768])
            nc.scalar.dma_start(c64[:, 768:872], c64_e[:, 768:872])
            xw = c64[:, 0:512]
            wqs_rep = c64[:, 512:768]
            wk40 = c64[:, 768:808]
            wv = c64[:, 808:872]
            c128 = cst.tile([128, 833], f32r)
            nc.sync.dma_start(c128[:], c128_e[:])
            awT4w = c128[:, 512:768]
            e65m = cst.tile([1, C + 1], bf16)
            nc.vector.tensor_copy(e65m[:], c128[0:1, 768:833])

            zb = cst.tile([128, 1], f32)
            nc.vector.memset(zb[:], 0.0)
            ones64f = cst.tile([1, C], f32)
            nc.vector.memset(ones64f[:], 1.0)
            ones64 = cst.tile([1, C], f32r)
            nc.vector.tensor_copy(ones64[:], ones64f[:])
            wact_o = cst.tile([128, 1], f32)

            # ---- working SBUF ----
            qT9 = sbp.tile([128, 1024], f32r)    # 4 x [128, 256] proj chunks
            qr9 = sbp.tile([128, 420], f32r)     # 4 x [128, 105] row-interp
            qf9 = sbp.tile([128, NQ], f32r)      # (d,S)-replicated upsampled q
            k4 = sbp.tile([40, 512], f32r)       # 2 score row-blocks of keys
            vTa = sbp.tile([128, 8 * 96], bf16)
            minf = sbp.tile([1, NQ], bf16)
            dent = sbp.tile([32, NQ], f32)       # 32x32-transposed denom
            rdsrc = sbp.tile([32, NQ], f32)      # recip, still transposed
            rdrow = sbp.tile([32, NQ], f32)      # row 0 = 1/denom
            rrow_r = sbp.tile([1, NQ], f32r)     # f32r copy for the bc matmul
            num_sb = sbp.tile([C, NQ], f32)
            fin = sbp.tile([C, NQ], f32)

            nc.vector.memset(vTa[:], 1.0)  # col 64 of each 96 = denom ones
            nc.vector.memset(rdsrc[:], 1.0)  # junk slots read by T2

            # ACT exp-table preload (runs before DMAs land)
            nc.scalar.activation(wact_o[:], zb[:], EXP, bias=zb[:])

            # ---- projections ----
            with (
                tc.tile_pool(name="ps_q", bufs=1, space="PSUM") as psq,
                tc.tile_pool(name="ps_kv", bufs=1, space="PSUM") as pskv,
            ):
                # PE order: k/v (gated only on xb, which lands first)
                # then the q chain.
                # k in 2 score row-blocks: block r holds chunks {2c+r} at
                # free 128c.  Odd chunks via M=40 (rows 32:40) first, even
                # via M=8 overwrite rows 0:8.
                k_ps = pskv.tile([40, 512], f32, tag="kps")
                for c in range(4):
                    fsl = slice(128 * c, 128 * (c + 1))
                    nc.tensor.matmul(
                        k_ps[0:40, fsl], wk40,
                        xb[:, 128 * (2 * c + 1) : 128 * (2 * c + 2)],
                        start=True, stop=True, skip_group_check=True,
                    )
                    nc.tensor.matmul(
                        k_ps[0:8, fsl], wk40[:, 32:40],
                        xb[:, 128 * (2 * c) : 128 * (2 * c + 1)],
                        start=True, stop=True, skip_group_check=True,
                    )
                vt_ps = pskv.tile([128, 512], f32, tag="vt")
                for t in range(8):
                    nc.tensor.matmul(
                        vt_ps[:, 64 * t : 64 * (t + 1)],
                        xb[:, 128 * t : 128 * (t + 1)],
                        wv,
                        start=True,
                        stop=True,
                        skip_group_check=True,
                    )
                qT9_ps = psq.tile([128, 1024], f32, tag="qT")
                for s in range(4):
                    nc.tensor.matmul(
                        qT9_ps[:, 256 * s : 256 * (s + 1)],
                        xw[:, 128 * s : 128 * (s + 1)],
                        wqs_rep,
                        start=True,
                        stop=True,
                        skip_group_check=True,
                    )

                for s in range(4):
                    if s % 2:
                        nc.scalar.copy(
                            qT9[:, 256 * s : 256 * (s + 1)],
                            qT9_ps[:, 256 * s : 256 * (s + 1)],
                        )
                    else:
                        nc.vector.tensor_copy(
                            qT9[:, 256 * s : 256 * (s + 1)],
                            qT9_ps[:, 256 * s : 256 * (s + 1)],
                        )

                # ---- row interp: one kron matmul per 4-row slice ----
                qr9_ps = psq.tile([128, 1024], f32, tag="qr")
                for s in range(4):
                    nc.tensor.matmul(
                        qr9_ps[:, 256 * s : 256 * (s + 1)],
                        c128[:, 128 * s : 128 * (s + 1)],
                        qT9[:, 256 * s : 256 * (s + 1)],
                        start=True,
                        stop=True,
                        skip_group_check=True,
                    )
                for s in range(4):
                    if s % 2:
                        nc.scalar.copy(
                            qr9[:, 105 * s : 105 * (s + 1)],
                            qr9_ps[:, 256 * s : 256 * s + 105],
                        )
                    else:
                        nc.vector.tensor_copy(
                            qr9[:, 105 * s : 105 * (s + 1)],
                            qr9_ps[:, 256 * s : 256 * s + 105],
                        )

                nc.vector.tensor_copy(k4[0:8, :], k_ps[0:8, :])
                nc.scalar.copy(k4[32:40, :], k_ps[32:40, :])
                nc.vector.tensor_copy(
                    vTa[:].rearrange("p (t c) -> p t c", t=8)[:, :, 0:C],
                    vt_ps[:].rearrange("p (t c) -> p t c", t=8),
                )
                # rows 65:95 of out_ps become junk denom copies; they only
                # feed the never-read slots of the 32x32 transpose

            # ---- col interp: block-diag awT4w, one matmul per slice ----
            with tc.tile_pool(name="ps_f", bufs=2, space="PSUM") as psf:
                for s in range(4):
                    qf_ps = psf.tile([105, 256], f32, tag="qf")
                    nc.tensor.matmul(
                        qf_ps[:],
                        qr9[:, 105 * s : 105 * (s + 1)],
                        awT4w,
                        start=True,
                        stop=True,
                        skip_group_check=True,
                    )
                    if s % 2:
                        nc.scalar.copy(
                            qf9[0:105, 256 * s : 256 * (s + 1)], qf_ps[:]
                        )
                    else:
                        nc.vector.tensor_copy(
                            qf9[0:105, 256 * s : 256 * (s + 1)], qf_ps[:]
                        )

            # masked-row term from the S channel (row 64: block 2 slot 0 —
            # 32-aligned partition base, required by the DVE)
            nc.vector.tensor_scalar(
                minf[:], qf9[64:65, :], 0.0, MASK_BIG,
                mybir.AluOpType.is_le, mybir.AluOpType.mult,
            )

            # ---- main: scores (2-way row-packed), exp, PV accumulate ----
            with (
                tc.tile_pool(name="ps_o", bufs=1, space="PSUM") as pso,
                tc.tile_pool(name="ps_s0", bufs=1, space="PSUM") as pss0,
                tc.tile_pool(name="ps_s1", bufs=1, space="PSUM") as pss1,
            ):
                # rows 65:96 are junk, read only by the 32x32 denom transpose
                out_ps = pso.tile([96, NQ], f32)
                pss = [pss0, pss1]
                for c in range(4):
                    sT = []
                    for r in range(2):
                        sT_r = pss[r].tile([128, NQ], f32, tag=f"s{r}")
                        sT.append(sT_r)
                    for h in range(2):
                        for r in range(2):
                            nc.tensor.matmul(
                                sT[r][:, 512 * h : 512 * (h + 1)],
                                k4[32 * r : 32 * r + 8,
                                   128 * c : 128 * (c + 1)],
                                qf9[32 * r : 32 * r + 8,
                                    512 * h : 512 * (h + 1)],
                                start=True,
                                stop=True,
                                skip_group_check=True,
                                tile_position=(32 * r, 0),
                            )
                    for r in range(2):
                        t = 2 * c + r
                        pT = pexp.tile([128, NQ], bf16, tag="pT")
                        nc.scalar.activation(pT[:], sT[r][:], EXP, bias=zb[:])
                        for h in range(2):
                            nc.tensor.matmul(
                                out_ps[:, 512 * h : 512 * (h + 1)],
                                vTa[:, 96 * t : 96 * (t + 1)],
                                pT[:, 512 * h : 512 * (h + 1)],
                                start=(t == 0),
                                stop=False,
                                skip_group_check=True,
                            )
                for h in range(2):
                    nc.tensor.matmul(
                        out_ps[0 : C + 1, 512 * h : 512 * (h + 1)],
                        e65m[:],
                        minf[:, 512 * h : 512 * (h + 1)],
                        start=False,
                        stop=True,
                        skip_group_check=True,
                    )  # rows 65:95 keep plain denom (mask term only row 64)

                # ---- epilogue ----
                # The [1, 512] denom row reciprocal is lane-serial on the
                # DVE (6.5 ns/elem).  Instead: 32x32 stream-transpose the
                # denom row into 32 partitions, reciprocal 32-wide, and
                # stream-transpose back (row 0).
                with tc.tile_pool(name="ps_e", bufs=1, space="PSUM") as pse:
                    bc_ps = pse.tile([C, NQ], f32)
                    for h in (0, 1):
                        sl = slice(512 * h, 512 * (h + 1))
                        nc.vector.transpose(dent[:, sl], out_ps[C:C + 32, sl])
                        nc.vector.reciprocal(
                            rdsrc[:, sl].rearrange(
                                "p (b q) -> p b q", q=32
                            )[:, :, 0],
                            dent[:, sl].rearrange(
                                "p (b q) -> p b q", q=32
                            )[:, :, 0],
                        )
                        nc.vector.transpose(rdrow[:, sl], rdsrc[:, sl])
                        nc.scalar.copy(rrow_r[:, sl], rdrow[0:1, sl])
                        nc.tensor.matmul(
                            bc_ps[:, sl], ones64[:], rrow_r[:, sl],
                            start=True, stop=True, skip_group_check=True,
                        )
                        nc.scalar.copy(num_sb[:, sl], out_ps[0:C, sl])
                        nc.vector.tensor_mul(
                            fin[:, sl], num_sb[:, sl], bc_ps[:, sl]
                        )
                        if h:
                            nc.scalar.dma_start(out_e[:, sl], fin[:, sl])
                        else:
                            nc.sync.dma_start(out_e[:, sl], fin[:, sl])

    nc.finalize()
    return nc


_NC = None


def _get_nc():
    global _NC
    if _NC is None:
        _NC = _build_nc()
    return _NC


def _in_maps(x, Wq, Wk, Wv):
    x = np.asarray(x, np.float32)
    Wq = np.asarray(Wq, np.float32)
    Wk = np.asarray(Wk, np.float32)
    Wv = np.asarray(Wv, np.float32)

    wqs_rep = np.zeros((C, 256), np.float32)
    for r in range(4):
        if r == 2:
            wqs_rep[:, 32 * r] = Wq.sum(axis=0)  # S channel, 32-aligned
        else:
            wqs_rep[:, 32 * r : 32 * r + D] = Wq.T
    wk40 = np.zeros((C, 40), np.float32)
    wk40[:, 32:40] = Wk.T
    awT4w = _col_mat()  # [128, 256]

    e65row = np.zeros((128, 65), np.float32)
    e65row[0, C] = 1.0
    maps = []
    for i in range(N_CORES):
        b, S = divmod(i, 4)
        xb = np.ascontiguousarray(x[b].reshape(C, H * W))
        wins = _row_windows(S)
        xw = np.zeros((C, 512), np.float32)
        K_all = np.zeros((128, 512), np.float32)
        for s, (h0, A4) in enumerate(wins):
            xw[:, 128 * s : 128 * (s + 1)] = x[b][:, h0 : h0 + 4, :].reshape(
                C, 128
            )
            K_all[:, 128 * s : 128 * (s + 1)] = np.kron(A4.T, np.eye(32))
        c64 = np.concatenate([xw, wqs_rep, wk40, Wv.T], axis=1)  # [64, 872]
        c128 = np.concatenate([K_all, awT4w, e65row], axis=1)  # [128, 833]
        maps.append(
            {
                "xb": xb,
                "c64": np.ascontiguousarray(c64),
                "c128": np.ascontiguousarray(c128),
            }
        )
    return maps


def _run(x, Wq, Wk, Wv, trace=False):
    from concourse.bass_utils import run_bass_kernel_spmd

    nc = _get_nc()
    res = run_bass_kernel_spmd(
        nc, _in_maps(x, Wq, Wk, Wv), core_ids=list(range(N_CORES)), trace=trace
    )
    out = np.empty((B, C, 4 * H * W), np.float32)
    for i in range(N_CORES):
        b, s = divmod(i, 4)
        out[b, :, s * NQ : (s + 1) * NQ] = res.results[i]["out"]
    return out.reshape(B, C, 2 * W, 2 * H), res


def kernel(x, Wq, Wk, Wv):
    out, _ = _run(x, Wq, Wk, Wv)
    return out
